# revision 1
# baseline (speedup 1.0000x reference)
"""DeformConv2d forward on 8 Trainium2 NeuronCores (Bass/Tile).

x[8,128,96,96] f32, offset[8,18,96,96] f32, weight[128,128,3,3] f32
-> out[8,128,96,96] f32. Deformable 3x3 conv, pad 1, stride 1, bilinear
sampling with zero padding. Data-parallel over batch: one element per core.

Per-core pipeline:
  A. x -> fp16, PE-transpose -> x_t[9216,128] in DRAM (pixel-major).
     conv weight -> fp16, PE-transpose -> WkT[ci, k*128+co].
  B. offsets PE-transposed into a position-packed layout [128, 72*18].
  C. DVE index/weight math in a [128, 9*72] packed layout (pos = c*128+p):
     bilinear corner weights A0,A1,B0,B1 (fp16) and pair-row indices
     IDXT, IDXB (top = y0c*96+x0c, bottom = (y1c)*96+x0c, both always
     in-range; out-of-image corners get zero weight).
  D. PE-transpose those to j-ordered DRAM rows (w_rows[36], idx_rows[18]).
  E. idx_rows -> 16-partition-wrapped SBUF layout for dma_gather;
     w_rows -> W36 SBUF.
  F. Per (chunk of 1024 positions, tap): dma_gather pulls (x0,x0+1) fp16
     pixel-pair columns for top and bottom rows (channels on partitions);
     PE broadcasts the 4 slot-weight rows across partitions (ones-matmul
     into PSUM), ACT evacuates to fp16; DVE multiplies gathered pairs by
     slot weights; PE GEMM accumulates over (ci, tap, slot) in PSUM.
"""
import sys
if '/opt/trn_rl_repo' not in sys.path:
    sys.path.insert(0, '/opt/trn_rl_repo')

import numpy as np

import concourse.bacc as bacc_mod
import concourse.mybir as mybir
import concourse.tile as tile
from concourse.ap import AP

f32 = mybir.dt.float32
f16 = mybir.dt.float16
i16 = mybir.dt.int16
i32 = mybir.dt.int32
Alu = mybir.AluOpType

P = 128
H = W = 96
NPOS = H * W              # 9216
NT = NPOS // P            # 72 position tiles
K = 9
NROW = NPOS - 1           # pair windows in x_t
CW = 1536                 # main-loop position chunk
GW = 768                  # per-gather-instruction indices (hw limit <= 896)
NCH = NPOS // CW          # 6 chunks
SUB = 512                 # GEMM moving sub-chunk


def _h(ap_or_handle):
    return ap_or_handle.tensor if hasattr(ap_or_handle, 'tensor') else ap_or_handle


import os
STAGE = os.environ.get("KSTAGE", "F")


def build_nc():
    nc = bacc_mod.Bacc()
    x_in = nc.declare_dram_parameter("x", [P, NPOS], f32, isOutput=False)
    off_in = nc.declare_dram_parameter("offset", [18, NPOS], f32, isOutput=False)
    w_in = nc.declare_dram_parameter("weight", [P, 1152], f32, isOutput=False)
    out = nc.declare_dram_parameter("out", [P, NPOS], f32, isOutput=True)

    with tile.TileContext(nc) as tc:
        with tc.tile_pool(name="const", bufs=1) as cpool, \
             tc.tile_pool(name="persist", bufs=1) as ppool, \
             tc.tile_pool(name="dram", bufs=1, space="DRAM") as dpool:
            x_t = dpool.tile([NPOS, P], f16, name="x_t")
            w_rows = dpool.tile([36, NPOS], f16, name="w_rows")
            idx_rows = dpool.tile([18, NPOS], i16, name="idx_rows")
            # ---------- constants ----------
            ident16 = cpool.tile([P, P], f16)
            ident32 = cpool.tile([P, P], f32)
            ones_row = cpool.tile([1, P], f16)
            nc.vector.memset(ones_row[:], 1.0)
            onesP = cpool.tile([P, P], f32)
            nc.vector.memset(onesP[:], 1.0)
            ramp128 = cpool.tile([P, P], f32)
            nc.vector.tensor_tensor_scan(ramp128[:], onesP[:], onesP[:], -1.0,
                                         Alu.mult, Alu.add)
            # pcol[p] = p via DRAM bounce (partition-spread load)
            pcol_d = dpool.tile([1, P], f32, name="pcol_d")
            nc.sync.dma_start(pcol_d[:], ramp128[0:1, :])
            pcol = cpool.tile([P, 1], f32)
            src_p = AP(tensor=_h(pcol_d), offset=0, ap=[[1, P], [1, 1]])
            nc.sync.dma_start(pcol[:], src_p)
            nc.vector.tensor_scalar(ident32[:], ramp128[:], pcol[:], None,
                                    Alu.is_equal)
            nc.vector.tensor_copy(ident16[:], ident32[:])

            if STAGE == "K2":
                zk = ppool.tile([P, 3 * P], f32, name="zk")
                nc.vector.tensor_copy(zk[:, 0:P], ident32[:])
                nc.vector.tensor_copy(zk[:, P:2 * P], ramp128[:])
                nc.vector.tensor_copy(zk[:, 2 * P:2 * P + 1], pcol[:])
                nc.sync.dma_start(out[:, 0:3 * P], zk[:])
                zk2 = ppool.tile([P, NPOS - 3 * P], f32, name="zk2")
                nc.vector.memset(zk2[:], 0.0)
                nc.sync.dma_start(out[:, 3 * P:], zk2[:])
            # ---------- persistent tiles ----------
            idxw = ppool.tile([P, 18 * 576], i16)
            WkT = ppool.tile([P, K * P], f16)

            # ---------- phase A: x -> x_t, weights -> WkT ----------
            with tc.tile_pool(name="prepA", bufs=2) as pa, \
                 tc.tile_pool(name="prepAp", bufs=3, space="PSUM") as pap:
                x_sb = pa.tile([P, NPOS], f32, tag="xsb")
                nc.sync.dma_start(x_sb[:], x_in[:])
                x16 = pa.tile([P, NPOS], f16, tag="x16")
                nc.scalar.copy(x16[:], x_sb[:])
                for tq in range(NT // 4):
                    pt4 = pap.tile([P, 4 * P], f16, tag="pt4")
                    for j in range(4):
                        t = tq * 4 + j
                        nc.tensor.transpose(pt4[:, j * P:(j + 1) * P],
                                            x16[:, t * P:(t + 1) * P], ident16[:])
                    ev = pa.tile([P, 4 * P], f16, tag="ev")
                    nc.scalar.copy(ev[:], pt4[:])
                    dst = AP(tensor=_h(x_t), offset=tq * 512 * P,
                             ap=[[P, P], [128 * P, 4], [1, P]])
                    nc.sync.dma_start(dst, ev[:].rearrange("r (j c) -> r j c", j=4))

                w_sb = pa.tile([P, 1152], f32, tag="wsb")
                nc.sync.dma_start(w_sb[:], w_in[:])
                w16 = pa.tile([P, 1152], f16, tag="w16")
                nc.scalar.copy(w16[:], w_sb[:])
                for k in range(K):
                    wkc = pa.tile([P, P], f16, tag="wkc")
                    nc.vector.tensor_copy(wkc[:], w16[:, k:1152:9])
                    ptw = pap.tile([P, P], f16, tag="ptw")
                    nc.tensor.transpose(ptw[:], wkc[:], ident16[:])
                    nc.scalar.copy(WkT[:, k * P:(k + 1) * P], ptw[:])

            # ---------- phases B-D ----------
            if STAGE == "A":
                zo = ppool.tile([P, NPOS], f32)
                nc.vector.memset(zo[:], 0.0)
                nc.sync.dma_start(out[:], zo[:])
            if STAGE != "A":
                with tc.tile_pool(name="prepB", bufs=1) as pb, \
                     tc.tile_pool(name="prepBp", bufs=2, space="PSUM") as pbp:
                    off_sb = pb.tile([18, NPOS], f32, tag="offsb")
                    nc.sync.dma_start(off_sb[:], off_in[:])
                    offt = pb.tile([P, NT * 18], f32, tag="offt")
                    for tg in range(3):
                        pso = pbp.tile([P, 24 * 18], f32, tag="pso")
                        for j in range(24):
                            t = tg * 24 + j
                            nc.tensor.transpose(pso[:, j * 18:(j + 1) * 18],
                                                off_sb[0:18, t * P:(t + 1) * P],
                                                ident32[0:18, 0:18])
                        nc.vector.tensor_copy(offt[:, tg * 432:(tg + 1) * 432], pso[:])

                    # ---------- phase C: math ----------
                    NF = K * NT  # 648

                    def mt(tag, dt=f32):
                        return pb.tile([P, NF], dt, tag=tag, name=tag)

                    posf = pb.tile([P, NT], f32, tag="posf")
                    nc.vector.tensor_scalar(posf[:], ramp128[:, 0:NT], 128.0, None,
                                            Alu.mult)
                    nc.vector.tensor_scalar(posf[:], posf[:], pcol[:], None, Alu.add)

                    q0i = pb.tile([P, NT], i32, tag="q0i")
                    tmpq = pb.tile([P, NT], f32, tag="tmpq")
                    nc.vector.tensor_scalar(tmpq[:], posf[:], 1.0 / 96.0, None, Alu.mult)
                    nc.vector.tensor_copy(q0i[:], tmpq[:])
                    q0 = pb.tile([P, NT], f32, tag="q0")
                    nc.vector.tensor_copy(q0[:], q0i[:])
                    r0 = pb.tile([P, NT], f32, tag="r0")
                    nc.vector.scalar_tensor_tensor(r0[:], q0[:], -96.0, posf[:],
                                                   Alu.mult, Alu.add)
                    ltz = pb.tile([P, NT], f32, tag="ltz")
                    nc.vector.tensor_scalar(ltz[:], r0[:], 0.0, None, Alu.is_lt)
                    gez = pb.tile([P, NT], f32, tag="gez")
                    nc.vector.tensor_scalar(gez[:], r0[:], 96.0, None, Alu.is_ge)
                    Rr = pb.tile([P, NT], f32, tag="Rr")
                    nc.vector.tensor_tensor(Rr[:], q0[:], ltz[:], Alu.subtract)
                    nc.vector.tensor_tensor(Rr[:], Rr[:], gez[:], Alu.add)
                    Cc = pb.tile([P, NT], f32, tag="Cc")
                    nc.vector.scalar_tensor_tensor(Cc[:], ltz[:], 96.0, r0[:],
                                                   Alu.mult, Alu.add)
                    nc.vector.scalar_tensor_tensor(Cc[:], gez[:], -96.0, Cc[:],
                                                   Alu.mult, Alu.add)

                    BY = mt("BY")
                    BX = mt("BX")
                    for k in range(K):
                        ky, kx = k // 3, k % 3
                        nc.vector.tensor_scalar(BY[:, k * NT:(k + 1) * NT], Rr[:],
                                                float(ky - 1), None, Alu.add)
                        nc.vector.tensor_scalar(BX[:, k * NT:(k + 1) * NT], Cc[:],
                                                float(kx - 1), None, Alu.add)

                    offv = offt[:].rearrange("p (t pl) -> p pl t", pl=18)
                    py = mt("py")
                    px = mt("px")
                    nc.vector.tensor_tensor(
                        py[:].rearrange("p (k t) -> p k t", k=K),
                        offv[:, 0:18:2, :],
                        BY[:].rearrange("p (k t) -> p k t", k=K), Alu.add)
                    nc.vector.tensor_tensor(
                        px[:].rearrange("p (k t) -> p k t", k=K),
                        offv[:, 1:18:2, :],
                        BX[:].rearrange("p (k t) -> p k t", k=K), Alu.add)

                    def floor_frac(v, pfx):
                        vi = mt(pfx + "i", i32)
                        nc.vector.tensor_copy(vi[:], v[:])
                        vf = mt(pfx + "f")
                        nc.vector.tensor_copy(vf[:], vi[:])
                        fr = mt(pfx + "fr")
                        nc.vector.tensor_tensor(fr[:], v[:], vf[:], Alu.subtract)
                        ng = mt(pfx + "ng")
                        nc.vector.tensor_scalar(ng[:], fr[:], 0.0, None, Alu.is_lt)
                        nc.vector.tensor_tensor(vf[:], vf[:], ng[:], Alu.subtract)
                        nc.vector.tensor_tensor(fr[:], fr[:], ng[:], Alu.add)
                        return vf, fr

                    y0, fy = floor_frac(py, "y")
                    x0, fx = floor_frac(px, "x")

                    def range_mask(v, lo, hi, pfx):
                        g = mt(pfx + "g")
                        nc.vector.tensor_scalar(g[:], v[:], float(lo), None, Alu.is_ge)
                        l = mt(pfx + "l")
                        nc.vector.tensor_scalar(l[:], v[:], float(hi), None, Alu.is_le)
                        nc.vector.tensor_tensor(g[:], g[:], l[:], Alu.mult)
                        return g

                    vt = range_mask(y0, 0, 95, "vt")
                    vb = range_mask(y0, -1, 94, "vb")
                    inr = range_mask(x0, 0, 94, "inr")
                    omfy = mt("omfy")
                    nc.vector.tensor_scalar(omfy[:], fy[:], -1.0, 1.0, Alu.mult, Alu.add)
                    omfx = mt("omfx")
                    nc.vector.tensor_scalar(omfx[:], fx[:], -1.0, 1.0, Alu.mult, Alu.add)
                    wtop = mt("wtop")
                    nc.vector.tensor_tensor(wtop[:], omfy[:], vt[:], Alu.mult)
                    wbot = mt("wbot")
                    nc.vector.tensor_tensor(wbot[:], fy[:], vb[:], Alu.mult)
                    em1 = mt("em1")
                    nc.vector.tensor_scalar(em1[:], x0[:], -1.0, None, Alu.is_equal)
                    e95 = mt("e95")
                    nc.vector.tensor_scalar(e95[:], x0[:], 95.0, None, Alu.is_equal)
                    s0 = mt("s0")
                    s1 = mt("s1")
                    tmp = mt("tmp")
                    nc.vector.tensor_tensor(s0[:], inr[:], omfx[:], Alu.mult)
                    nc.vector.tensor_tensor(tmp[:], em1[:], fx[:], Alu.mult)
                    nc.vector.tensor_tensor(s0[:], s0[:], tmp[:], Alu.add)
                    nc.vector.tensor_tensor(s1[:], inr[:], fx[:], Alu.mult)
                    nc.vector.tensor_tensor(tmp[:], e95[:], omfx[:], Alu.mult)
                    nc.vector.tensor_tensor(s1[:], s1[:], tmp[:], Alu.add)

                    A0 = mt("A0", f16)
                    A1 = mt("A1", f16)
                    B0 = mt("B0", f16)
                    B1 = mt("B1", f16)
                    nc.vector.tensor_tensor(A0[:], wtop[:], s0[:], Alu.mult)
                    nc.vector.tensor_tensor(A1[:], wtop[:], s1[:], Alu.mult)
                    nc.vector.tensor_tensor(B0[:], wbot[:], s0[:], Alu.mult)
                    nc.vector.tensor_tensor(B1[:], wbot[:], s1[:], Alu.mult)

                    x0c = mt("x0c")
                    nc.vector.tensor_scalar(x0c[:], x0[:], 0.0, 94.0, Alu.max, Alu.min)
                    y0c = mt("y0c")
                    nc.vector.tensor_scalar(y0c[:], y0[:], 0.0, 95.0, Alu.max, Alu.min)
                    y1p = mt("y1p")
                    nc.vector.tensor_scalar(y1p[:], y0[:], -1.0, 94.0, Alu.max, Alu.min)
                    x0c96 = mt("x0c96")
                    nc.vector.tensor_scalar(x0c96[:], x0c[:], 96.0, None, Alu.add)
                    IDXT = mt("IDXT")
                    nc.vector.scalar_tensor_tensor(IDXT[:], y0c[:], 96.0, x0c[:],
                                                   Alu.mult, Alu.add)
                    IDXB = mt("IDXB")
                    nc.vector.scalar_tensor_tensor(IDXB[:], y1p[:], 96.0, x0c96[:],
                                                   Alu.mult, Alu.add)

                    # ---------- phase D ----------
                    for k in range(K):
                        psw = pbp.tile([NT, 4 * P], f16, tag="psw")
                        for s, tt_ in enumerate((A0, A1, B0, B1)):
                            nc.tensor.transpose(psw[:, s * P:(s + 1) * P],
                                                tt_[:, k * NT:(k + 1) * NT],
                                                ident16[:])
                        evw = pb.tile([NT, 4 * P], f16, tag="evw")
                        nc.scalar.copy(evw[:], psw[:])
                        dstw = AP(tensor=_h(w_rows), offset=(4 * k) * NPOS,
                                  ap=[[P, NT], [NPOS, 4], [1, P]])
                        nc.sync.dma_start(dstw,
                                          evw[:].rearrange("c (s e) -> c s e", s=4))

                        psi = pbp.tile([NT, 2 * P], f32, tag="psi")
                        nc.tensor.transpose(psi[:, 0:P],
                                            IDXT[:, k * NT:(k + 1) * NT], ident32[:])
                        nc.tensor.transpose(psi[:, P:2 * P],
                                            IDXB[:, k * NT:(k + 1) * NT], ident32[:])
                        evi = pb.tile([NT, 2 * P], i16, tag="evi")
                        nc.vector.tensor_copy(evi[:], psi[:])
                        dsti = AP(tensor=_h(idx_rows), offset=(2 * k) * NPOS,
                                  ap=[[P, NT], [NPOS, 2], [1, P]])
                        nc.sync.dma_start(dsti,
                                          evi[:].rearrange("c (s e) -> c s e", s=2))

            if STAGE == "D2":
                ird = ppool.tile([18, NPOS], i16, name="ird")
                nc.sync.dma_start(ird[:], idx_rows[:])
                irf = ppool.tile([18, NPOS], f32, name="irf")
                nc.vector.tensor_copy(irf[:], ird[:])
                nc.sync.dma_start(out[0:18, :], irf[:])
            # ---------- phase E ----------
            if STAGE == "B":
                zo = ppool.tile([P, NPOS], f32)
                nc.vector.memset(zo[:], 0.0)
                nc.sync.dma_start(out[:], zo[:])
            if STAGE in ("E", "E2", "F1a", "F1b", "F1c", "F1", "F"):
                nc.vector.memset(idxw[:], 0)
                for q in range(18):
                    src = AP(tensor=_h(idx_rows), offset=q * NPOS,
                             ap=[[1, 16], [16, 576]])
                    nc.sync.dma_start(idxw[0:16, q * 576:(q + 1) * 576], src)
                nc.sync.dma_start(idxw[16:32, :], idxw[0:16, :])
                nc.sync.dma_start(idxw[32:64, :], idxw[0:32, :])
                nc.sync.dma_start(idxw[64:128, :], idxw[0:64, :])

            # ---------- phase F: main loop ----------
            if STAGE == "E2":
                zo2 = ppool.tile([P, NPOS], f32)
                nc.vector.tensor_copy(zo2[:], idxw[:, 0:NPOS])
                nc.sync.dma_start(out[:], zo2[:])
            if STAGE == "E":
                zo = ppool.tile([P, NPOS], f32)
                nc.vector.memset(zo[:], 0.0)
                nc.sync.dma_start(out[:], zo[:])
            n_chunks = NCH if STAGE == "F" else 1
            xt_win = AP(tensor=_h(x_t), offset=0, ap=[[P, NROW], [1, 2 * P]])
            if STAGE in ("F1", "F"):
                n_chunks = NCH if STAGE == "F" else 1
                xt_win = AP(tensor=_h(x_t), offset=0, ap=[[P, NROW], [1, 2 * P]])
                with tc.tile_pool(name="g", bufs=int(os.environ.get("GB", "4"))) as gp, \
                     tc.tile_pool(name="aw", bufs=int(os.environ.get("AB", "3"))) as awp, \
                     tc.tile_pool(name="c4", bufs=int(os.environ.get("CB", "3"))) as c4p, \
                     tc.tile_pool(name="ops", bufs=2) as osp, \
                     tc.tile_pool(name="awps", bufs=2, space="PSUM") as awps, \
                     tc.tile_pool(name="outps", bufs=int(os.environ.get("OB", "1")), space="PSUM") as outps:
                    for c in range(n_chunks):
                        out_ps = outps.tile([P, CW], f32, tag="ops", name="out_ps")
                        for k in range(K):
                            ghs = []
                            for h in range(CW // GW):
                                gTh = gp.tile([P, 2, GW], f16, tag="gT", name="gTh")
                                gBh = gp.tile([P, 2, GW], f16, tag="gB", name="gBh")
                                i0t = (2 * k) * 576 + (c * CW + h * GW) // 16
                                i0b = (2 * k + 1) * 576 + (c * CW + h * GW) // 16
                                qn = ((2 * k + h) % 8) if os.environ.get("QN") == "spread" else 0
                                nc.gpsimd.dma_gather(
                                    gTh[:], xt_win,
                                    idxw[:, i0t:i0t + GW // 16],
                                    num_idxs=GW, num_idxs_reg=GW,
                                    elem_size=2 * P, elem_step=P, transpose=True,
                                    queue_num=qn)
                                nc.gpsimd.dma_gather(
                                    gBh[:], xt_win,
                                    idxw[:, i0b:i0b + GW // 16],
                                    num_idxs=GW, num_idxs_reg=GW,
                                    elem_size=2 * P, elem_step=P, transpose=True,
                                    queue_num=(qn + 4) % 8 if qn else 0)
                                ghs.append((gTh, gBh))
                            wst = awp.tile([1, 4 * CW], f16, tag="wst")
                            wsrc = AP(tensor=_h(w_rows),
                                      offset=(4 * k) * NPOS + c * CW,
                                      ap=[[NPOS, 4], [1, CW]])
                            nc.sync.dma_start(
                                wst[:].rearrange("p (s e) -> p s e", s=4),
                                wsrc.unsqueeze(0))
                            aw = awp.tile([P, 4, CW], f16, tag="aw")
                            if os.environ.get("AW_MODE", "pe") == "dma":
                                awsrc = AP(tensor=_h(w_rows),
                                           offset=(4 * k) * NPOS + c * CW,
                                           ap=[[0, P], [NPOS, 4], [1, CW]])
                                nc.scalar.dma_start(aw[:], awsrc)
                            else:
                                for si in range(4):
                                    import os as _os
                                    if _os.environ.get("OB", "1") == "2":
                                        for g3 in range(CW // SUB):
                                            psb = awps.tile([P, SUB], f32, tag="awps",
                                                            name="psb")
                                            base = si * CW + g3 * SUB
                                            nc.tensor.matmul(
                                                psb[:], ones_row[:],
                                                wst[0:1, base:base + SUB],
                                                start=True, stop=True)
                                            if (si + g3) % 2 == 0:
                                                nc.scalar.copy(
                                                    aw[:, si, g3 * SUB:(g3 + 1) * SUB], psb[:])
                                            else:
                                                nc.vector.tensor_copy(
                                                    aw[:, si, g3 * SUB:(g3 + 1) * SUB], psb[:])
                                    else:
                                        for h in range(CW // GW):
                                            psb = awps.tile([P, GW], f32, tag="awps",
                                                            name="psb")
                                            base = si * CW + h * GW
                                            nc.tensor.matmul(
                                                psb[:, 0:SUB], ones_row[:],
                                                wst[0:1, base:base + SUB],
                                                start=True, stop=True)
                                            nc.tensor.matmul(
                                                psb[:, SUB:GW], ones_row[:],
                                                wst[0:1, base + SUB:base + GW],
                                                start=True, stop=True)
                                            if (si + h) % 2 == 0:
                                                nc.scalar.copy(
                                                    aw[:, si, h * GW:(h + 1) * GW], psb[:])
                                            else:
                                                nc.vector.tensor_copy(
                                                    aw[:, si, h * GW:(h + 1) * GW], psb[:])
                            c4 = c4p.tile([P, 4, CW], f16, tag="c4")
                            for h, (gTh, gBh) in enumerate(ghs):
                                hs = slice(h * GW, (h + 1) * GW)
                                nc.vector.tensor_tensor(
                                    c4[:, 0:2, hs], gTh[:],
                                    aw[:, 0:2, hs], Alu.mult)
                                nc.vector.tensor_tensor(
                                    c4[:, 2:4, hs], gBh[:],
                                    aw[:, 2:4, hs], Alu.mult)
                            for j in range(CW // SUB):
                                for si in range(4):
                                    nc.tensor.matmul(
                                        out_ps[:, j * SUB:(j + 1) * SUB],
                                        WkT[:, k * P:(k + 1) * P],
                                        c4[:, si, j * SUB:(j + 1) * SUB],
                                        start=(k == 0 and si == 0),
                                        stop=(k == K - 1 and si == 3),
                                        skip_group_check=True)
                        osb = osp.tile([P, CW], f32, tag="osb")
                        nc.vector.tensor_copy(osb[:], out_ps[:])
                        nc.sync.dma_start(out[:, c * CW:(c + 1) * CW], osb[:])
    nc.compile()
    return nc


_NC = None


def kernel(x, offset, weight):
    global _NC
    if _NC is None:
        _NC = build_nc()
    from concourse.bass_utils import run_bass_kernel_spmd
    B = x.shape[0]
    w2 = np.ascontiguousarray(weight.reshape(P, 1152)).astype(np.float32)
    in_maps = []
    for b in range(B):
        in_maps.append({
            "x": np.ascontiguousarray(np.asarray(x)[b].reshape(P, NPOS), dtype=np.float32),
            "offset": np.ascontiguousarray(np.asarray(offset)[b].reshape(18, NPOS), dtype=np.float32),
            "weight": w2,
        })
    res = run_bass_kernel_spmd(_NC, in_maps, list(range(B)))
    outs = [res.results[b]["out"].reshape(P, H, W) for b in range(B)]
    return np.stack(outs).astype(np.float32)



# revision 33
# speedup vs baseline: 1.4530x; 1.4530x over previous
"""DeformConv2d forward on 8 Trainium2 NeuronCores (Bass/Tile).

x[8,128,96,96] f32, offset[8,18,96,96] f32, weight[128,128,3,3] f32
-> out[8,128,96,96] f32. Deformable 3x3 conv, pad 1, stride 1, bilinear
sampling with zero padding. Data-parallel over batch: one element per core.

v3: overlapped prep (offsets loaded first; index/weight math split across
DVE and GPSIMD; per-tap idxw loads so gathers start as soon as x_t and the
first tap's indices are ready), swizzled idx evac for contiguous wrap
loads, ACT/Pool/DVE-balanced PSUM evacuation, direct fp16 premultiply.
"""
import sys
if '/opt/trn_rl_repo' not in sys.path:
    sys.path.insert(0, '/opt/trn_rl_repo')

import os
import numpy as np

import concourse.bacc as bacc_mod
import concourse.mybir as mybir
import concourse.tile as tile
from concourse.ap import AP

f32 = mybir.dt.float32
f16 = mybir.dt.float16
i16 = mybir.dt.int16
i32 = mybir.dt.int32
Alu = mybir.AluOpType

P = 128
H = W = 96
NPOS = H * W              # 9216
NT = NPOS // P            # 72 position tiles
K = 9
NROW = NPOS - 1           # pair windows in x_t
CW = 1536                 # main-loop position chunk
NCH = NPOS // CW          # 6 chunks
SUB = int(os.environ.get("SUB", "512"))  # GEMM moving sub-chunk
NEV_DVE = int(os.environ.get("NEV_DVE", "1"))   # aw evacs on DVE (of 6)
NEV_POOL = int(os.environ.get("NEV_POOL", "0"))  # aw evacs on Pool (of 6)
NFUSE = int(os.environ.get("NFUSE", "2"))   # psb chunks premultiplied from PSUM


def _h(ap_or_handle):
    return ap_or_handle.tensor if hasattr(ap_or_handle, 'tensor') else ap_or_handle


def build_nc():
    nc = bacc_mod.Bacc()
    x_in = nc.declare_dram_parameter("x", [P, NPOS], f32, isOutput=False)
    off_in = nc.declare_dram_parameter("offset", [18, NPOS], f32, isOutput=False)
    w_in = nc.declare_dram_parameter("weight", [P, 1152], f32, isOutput=False)
    out = nc.declare_dram_parameter("out", [P, NPOS], f32, isOutput=True)

    with tile.TileContext(nc) as tc:
        with tc.tile_pool(name="const", bufs=1) as cpool, \
             tc.tile_pool(name="persist", bufs=1) as ppool, \
             tc.tile_pool(name="dram", bufs=1, space="DRAM") as dpool:
            x_t = dpool.tile([NPOS, P], f16, name="x_t")
            w_rows = dpool.tile([36, NPOS], f16, name="w_rows")
            idx_d = dpool.tile([16, 18 * 576], i16, name="idx_d")
            # ---------- constants ----------
            ident16 = cpool.tile([P, P], f16)
            ident32 = cpool.tile([P, P], f32)
            ones_row = cpool.tile([1, P], f16)
            nc.vector.memset(ones_row[:], 1.0)
            onesP = cpool.tile([P, P], f32)
            nc.vector.memset(onesP[:], 1.0)
            ramp128 = cpool.tile([P, P], f32)
            nc.vector.tensor_tensor_scan(ramp128[:], onesP[:], onesP[:], -1.0,
                                         Alu.mult, Alu.add)
            pcol_d = dpool.tile([1, P], f32, name="pcol_d")
            nc.sync.dma_start(pcol_d[:], ramp128[0:1, :])
            pcol = cpool.tile([P, 1], f32)
            src_p = AP(tensor=_h(pcol_d), offset=0, ap=[[1, P], [1, 1]])
            nc.sync.dma_start(pcol[:], src_p)
            nc.vector.tensor_scalar(ident32[:], ramp128[:], pcol[:], None,
                                    Alu.is_equal)
            nc.vector.tensor_copy(ident16[:], ident32[:])

            # ---------- persistent tiles ----------
            # three idx groups: tap 0 alone (unblocks the first gather
            # early), taps 1-3, taps 4-8 (replicated while the loop runs)
            IDX_GROUPS = [(0, 1), (1, 4), (4, 9)]
            idxw_g = [ppool.tile([P, 1152 * (b - a)], i16, name=f"idxw_g{a}")
                      for a, b in IDX_GROUPS]
            WkT = ppool.tile([P, K * P], f16)

            with tc.tile_pool(name="prepA", bufs=2) as pa, \
                 tc.tile_pool(name="prepEv", bufs=2) as pev, \
                 tc.tile_pool(name="prepAp", bufs=2, space="PSUM") as pap, \
                 tc.tile_pool(name="prepB", bufs=1) as pb, \
                 tc.tile_pool(name="prepBp", bufs=2, space="PSUM") as pbp:
                # offsets + weights first: unblocks DVE/GPSIMD math and WkT
                # while the larger x load streams in behind them.
                off_sb = pb.tile([18, NPOS], f32, tag="offsb")
                nc.sync.dma_start(off_sb[:], off_in[:])
                w_sb = pa.tile([P, 1152], f32, tag="wsb")
                nc.sync.dma_start(w_sb[:], w_in[:])

                # ---------- phase B: offsets -> position-packed ----------
                offt = pb.tile([P, NT * 18], f32, tag="offt")
                for tg in range(3):
                    pso = pbp.tile([P, 24 * 18], f32, tag="pso")
                    for j in range(24):
                        t = tg * 24 + j
                        nc.tensor.transpose(pso[:, j * 18:(j + 1) * 18],
                                            off_sb[0:18, t * P:(t + 1) * P],
                                            ident32[0:18, 0:18])
                    nc.vector.tensor_copy(offt[:, tg * 432:(tg + 1) * 432], pso[:])

                # ---------- phase A: x -> x_t, weights -> WkT ----------
                x16 = pb.tile([P, NPOS], f16, tag="x16")
                for xs in range(4):
                    x_sl = pa.tile([P, NPOS // 4], f32, tag="xsl")
                    nc.sync.dma_start(
                        x_sl[:], x_in[:, xs * (NPOS // 4):(xs + 1) * (NPOS // 4)])
                    nc.scalar.copy(
                        x16[:, xs * (NPOS // 4):(xs + 1) * (NPOS // 4)], x_sl[:])
                for tg in range(3):
                    ev24 = pev.tile([P, 24 * P], f16, tag="ev24")
                    for q in range(3):
                        tq = tg * 3 + q
                        pt8 = pap.tile([P, 8 * P], f16, tag="pt8")
                        for j in range(8):
                            t = tq * 8 + j
                            nc.tensor.transpose(pt8[:, j * P:(j + 1) * P],
                                                x16[:, t * P:(t + 1) * P],
                                                ident16[:])
                        nc.scalar.copy(ev24[:, q * 1024:(q + 1) * 1024], pt8[:])
                    dst = AP(tensor=_h(x_t), offset=tg * 3072 * P,
                             ap=[[P, P], [128 * P, 24], [1, P]])
                    nc.sync.dma_start(dst,
                                      ev24[:].rearrange("r (j c) -> r j c", j=24))

                w16 = pa.tile([P, 1152], f16, tag="w16")
                nc.scalar.copy(w16[:], w_sb[:])
                for k in range(K):
                    wkc = pa.tile([P, P], f16, tag="wkc")
                    nc.vector.tensor_copy(wkc[:], w16[:, k:1152:9])
                    ptw = pap.tile([P, 8 * P], f16, tag="pt8")
                    nc.tensor.transpose(ptw[:, 0:P], wkc[:], ident16[:])
                    nc.scalar.copy(WkT[:, k * P:(k + 1) * P], ptw[:, 0:P])

                # ---------- phase C: math (y-chain on DVE, x-chain on Pool) --
                NF = K * NT  # 648

                def mt(tag, dt=f32):
                    return pb.tile([P, NF], dt, tag=tag, name=tag)

                posf = pb.tile([P, NT], f32, tag="posf")
                nc.vector.tensor_scalar(posf[:], ramp128[:, 0:NT], 128.0, None,
                                        Alu.mult)
                nc.vector.tensor_scalar(posf[:], posf[:], pcol[:], None, Alu.add)

                q0i = pb.tile([P, NT], i32, tag="q0i")
                tmpq = pb.tile([P, NT], f32, tag="tmpq")
                nc.vector.tensor_scalar(tmpq[:], posf[:], 1.0 / 96.0, None, Alu.mult)
                nc.vector.tensor_copy(q0i[:], tmpq[:])
                q0 = pb.tile([P, NT], f32, tag="q0")
                nc.vector.tensor_copy(q0[:], q0i[:])
                r0 = pb.tile([P, NT], f32, tag="r0")
                nc.vector.scalar_tensor_tensor(r0[:], q0[:], -96.0, posf[:],
                                               Alu.mult, Alu.add)
                ltz = pb.tile([P, NT], f32, tag="ltz")
                nc.vector.tensor_scalar(ltz[:], r0[:], 0.0, None, Alu.is_lt)
                gez = pb.tile([P, NT], f32, tag="gez")
                nc.vector.tensor_scalar(gez[:], r0[:], 96.0, None, Alu.is_ge)
                Rr = pb.tile([P, NT], f32, tag="Rr")
                nc.vector.tensor_tensor(Rr[:], q0[:], ltz[:], Alu.subtract)
                nc.vector.tensor_tensor(Rr[:], Rr[:], gez[:], Alu.add)
                Cc = pb.tile([P, NT], f32, tag="Cc")
                nc.vector.scalar_tensor_tensor(Cc[:], ltz[:], 96.0, r0[:],
                                               Alu.mult, Alu.add)
                nc.vector.scalar_tensor_tensor(Cc[:], gez[:], -96.0, Cc[:],
                                               Alu.mult, Alu.add)

                BY = mt("BY", f16)
                BX = mt("BX", f16)
                for k in range(K):
                    ky, kx = k // 3, k % 3
                    nc.vector.tensor_scalar(BY[:, k * NT:(k + 1) * NT], Rr[:],
                                            float(ky - 1), None, Alu.add)
                    nc.gpsimd.tensor_scalar(BX[:, k * NT:(k + 1) * NT], Cc[:],
                                            float(kx - 1), None, Alu.add)

                offv = offt[:].rearrange("p (t pl) -> p pl t", pl=18)
                py = mt("py")
                px = mt("px")
                nc.vector.tensor_tensor(
                    py[:].rearrange("p (k t) -> p k t", k=K),
                    offv[:, 0:18:2, :],
                    BY[:].rearrange("p (k t) -> p k t", k=K), Alu.add)
                nc.gpsimd.tensor_tensor(
                    px[:].rearrange("p (k t) -> p k t", k=K),
                    offv[:, 1:18:2, :],
                    BX[:].rearrange("p (k t) -> p k t", k=K), Alu.add)

                def floor_frac(eng, v, pfx):
                    vi = mt(pfx + "i", i16)
                    eng.tensor_copy(vi[:], v[:])
                    vf = mt(pfx + "f")
                    eng.tensor_copy(vf[:], vi[:])
                    fr = mt(pfx + "fr")
                    eng.tensor_tensor(fr[:], v[:], vf[:], Alu.subtract)
                    ng = mt(pfx + "ng")
                    eng.tensor_scalar(ng[:], fr[:], 0.0, None, Alu.is_lt)
                    eng.tensor_tensor(vf[:], vf[:], ng[:], Alu.subtract)
                    eng.tensor_tensor(fr[:], fr[:], ng[:], Alu.add)
                    return vf, fr

                y0, fy = floor_frac(nc.vector, py, "y")
                x0, fx = floor_frac(nc.gpsimd, px, "x")

                def range_mask(eng, v, lo, hi, pfx):
                    g = mt(pfx + "g")
                    eng.tensor_scalar(g[:], v[:], float(lo), None, Alu.is_ge)
                    l = mt(pfx + "l")
                    eng.tensor_scalar(l[:], v[:], float(hi), None, Alu.is_le)
                    eng.tensor_tensor(g[:], g[:], l[:], Alu.mult)
                    return g

                vt = range_mask(nc.vector, y0, 0, 95, "vt")
                vb = range_mask(nc.vector, y0, -1, 94, "vb")
                inr = range_mask(nc.gpsimd, x0, 0, 94, "inr")
                omfy = mt("omfy", f16)
                nc.vector.tensor_scalar(omfy[:], fy[:], -1.0, 1.0, Alu.mult, Alu.add)
                omfx = mt("omfx", f16)
                nc.gpsimd.tensor_scalar(omfx[:], fx[:], -1.0, 1.0, Alu.mult, Alu.add)
                wtop = mt("wtop", f16)
                nc.vector.tensor_tensor(wtop[:], omfy[:], vt[:], Alu.mult)
                wbot = mt("wbot", f16)
                nc.vector.tensor_tensor(wbot[:], fy[:], vb[:], Alu.mult)
                em1 = mt("em1")
                nc.vector.tensor_scalar(em1[:], x0[:], -1.0, None, Alu.is_equal)
                e95 = mt("e95")
                nc.vector.tensor_scalar(e95[:], x0[:], 95.0, None, Alu.is_equal)
                s0 = mt("s0", f16)
                s1 = mt("s1", f16)
                tmp = mt("tmp", f16)
                nc.gpsimd.tensor_tensor(s0[:], inr[:], omfx[:], Alu.mult)
                nc.gpsimd.tensor_tensor(tmp[:], em1[:], fx[:], Alu.mult)
                nc.gpsimd.tensor_tensor(s0[:], s0[:], tmp[:], Alu.add)
                nc.gpsimd.tensor_tensor(s1[:], inr[:], fx[:], Alu.mult)
                nc.gpsimd.tensor_tensor(tmp[:], e95[:], omfx[:], Alu.mult)
                nc.gpsimd.tensor_tensor(s1[:], s1[:], tmp[:], Alu.add)

                A0 = mt("A0", f16)
                A1 = mt("A1", f16)
                B0 = mt("B0", f16)
                B1 = mt("B1", f16)
                nc.vector.tensor_tensor(A0[:], wtop[:], s0[:], Alu.mult)
                nc.vector.tensor_tensor(A1[:], wtop[:], s1[:], Alu.mult)
                nc.vector.tensor_tensor(B0[:], wbot[:], s0[:], Alu.mult)
                nc.vector.tensor_tensor(B1[:], wbot[:], s1[:], Alu.mult)

                x0c = mt("x0c")
                nc.vector.tensor_scalar(x0c[:], x0[:], 0.0, 94.0, Alu.max, Alu.min)
                y0c = mt("y0c")
                nc.vector.tensor_scalar(y0c[:], y0[:], 0.0, 95.0, Alu.max, Alu.min)
                y1p = mt("y1p")
                nc.vector.tensor_scalar(y1p[:], y0[:], -1.0, 94.0, Alu.max, Alu.min)
                x0c96 = mt("x0c96")
                nc.vector.tensor_scalar(x0c96[:], x0c[:], 96.0, None, Alu.add)
                IDXT = mt("IDXT")
                nc.vector.scalar_tensor_tensor(IDXT[:], y0c[:], 96.0, x0c[:],
                                               Alu.mult, Alu.add)
                IDXB = mt("IDXB")
                nc.vector.scalar_tensor_tensor(IDXB[:], y1p[:], 96.0, x0c96[:],
                                               Alu.mult, Alu.add)

                # ---------- phase D + per-tap idxw load ----------
                for k in range(K):
                    psw = pbp.tile([NT, 4 * P], f16, tag="psw")
                    for s, tt_ in enumerate((A0, A1, B0, B1)):
                        nc.tensor.transpose(psw[:, s * P:(s + 1) * P],
                                            tt_[:, k * NT:(k + 1) * NT],
                                            ident16[:])
                    evw = pb.tile([NT, 4 * P], f16, tag="evw")
                    nc.scalar.copy(evw[:], psw[:])
                    dstw = AP(tensor=_h(w_rows), offset=(4 * k) * NPOS,
                              ap=[[P, NT], [NPOS, 4], [1, P]])
                    nc.sync.dma_start(dstw,
                                      evw[:].rearrange("c (s e) -> c s e", s=4))

                    psi = pbp.tile([NT, 2 * P], f32, tag="psi")
                    nc.tensor.transpose(psi[:, 0:P],
                                        IDXT[:, k * NT:(k + 1) * NT], ident32[:])
                    nc.tensor.transpose(psi[:, P:2 * P],
                                        IDXB[:, k * NT:(k + 1) * NT], ident32[:])
                    # 16-wrap swizzle on evac: evi[t, h*128 + a*8 + b] =
                    # psi[t, h*128 + b*16 + a]  (a = pix%16, b = (pix%128)//16)
                    evi = pb.tile([NT, 2 * P], i16, tag="evi")
                    nc.vector.tensor_copy(
                        evi[:].rearrange("t (h a b) -> t h a b", h=2, a=16),
                        psi[:].rearrange("t (h b a) -> t h a b", h=2, b=8))
                    # idx_d[a][2k+h][t*8+b] — contiguous wrap-load layout
                    for hh in range(2):
                        dsti = AP(tensor=_h(idx_d), offset=(2 * k + hh) * 576,
                                  ap=[[8, NT], [18 * 576, 16], [1, 8]])
                        nc.sync.dma_start(
                            dsti,
                            evi[:, hh * P:(hh + 1) * P].rearrange(
                                "t (a b) -> t a b", a=16))
                    # wrap-load + replicate once per idx group
                    for gi, (ga, gb) in enumerate(IDX_GROUPS):
                        if k != gb - 1:
                            continue
                        ixg = idxw_g[gi]
                        srcw = AP(tensor=_h(idx_d), offset=(2 * ga) * 576,
                                  ap=[[18 * 576, 16], [1, 1152 * (gb - ga)]])
                        nc.scalar.dma_start(ixg[0:16, :], srcw)
                        nc.scalar.dma_start(ixg[16:32, :], ixg[0:16, :])
                        nc.scalar.dma_start(ixg[32:64, :], ixg[0:32, :])
                        nc.scalar.dma_start(ixg[64:128, :], ixg[0:64, :])

            # ---------- phase F: main loop ----------
            xt_win = AP(tensor=_h(x_t), offset=0, ap=[[P, NROW], [1, 2 * P]])
            with tc.tile_pool(name="g", bufs=4) as gp, \
                 tc.tile_pool(name="aw", bufs=2) as awp, \
                 tc.tile_pool(name="c4", bufs=3) as c4p, \
                 tc.tile_pool(name="wstp", bufs=2) as wsp, \
                 tc.tile_pool(name="ops", bufs=2) as osp, \
                 tc.tile_pool(name="awps", bufs=2, space="PSUM") as awps, \
                 tc.tile_pool(name="outps", bufs=1, space="PSUM") as outps:
                for c in range(NCH):
                    out_ps = outps.tile([P, CW], f32, tag="ops", name="out_ps")
                    c4_prev = []
                    for k in range(K):
                        # DGE inflight ring caps a transpose-gather at
                        # num_idxs*2/16+2 < 128 descriptors per DMA engine,
                        # i.e. num_idxs <= 896 — gather in GW=768 pieces.
                        GW = 768
                        ghs = []
                        for h in range(CW // GW):
                            gTh = gp.tile([P, 2, GW], f16, tag="gT", name="gTh")
                            gBh = gp.tile([P, 2, GW], f16, tag="gB", name="gBh")
                            gi = 0 if k == 0 else (1 if k < 4 else 2)
                            ka = k - IDX_GROUPS[gi][0]
                            i0t = ka * 1152 + (c * CW + h * GW) // 16
                            i0b = ka * 1152 + 576 + (c * CW + h * GW) // 16
                            nc.gpsimd.dma_gather(
                                gTh[:], xt_win,
                                idxw_g[gi][:, i0t:i0t + GW // 16],
                                num_idxs=GW, num_idxs_reg=GW,
                                elem_size=2 * P, elem_step=P, transpose=True)
                            nc.gpsimd.dma_gather(
                                gBh[:], xt_win,
                                idxw_g[gi][:, i0b:i0b + GW // 16],
                                num_idxs=GW, num_idxs_reg=GW,
                                elem_size=2 * P, elem_step=P, transpose=True)
                            ghs.append((gTh, gBh))
                        wst = wsp.tile([1, 4 * CW], f16, tag="wst")
                        wsrc = AP(tensor=_h(w_rows),
                                  offset=(4 * k) * NPOS + c * CW,
                                  ap=[[NPOS, 4], [1, CW]])
                        nc.sync.dma_start(
                            wst[:].rearrange("p (s e) -> p s e", s=4),
                            wsrc.unsqueeze(0))
                        aw = awp.tile([P, 4, CW], f16, tag="aw")
                        awf = aw[:].rearrange("p s c -> p (s c)")
                        for e in range(6):
                            psb = awps.tile([P, 1024], f32, tag="psb", name="psb")
                            base = e * 1024
                            nc.tensor.matmul(psb[:, 0:SUB], ones_row[:],
                                             wst[0:1, base:base + SUB],
                                             start=True, stop=True)
                            nc.tensor.matmul(psb[:, SUB:1024], ones_row[:],
                                             wst[0:1, base + SUB:base + 1024],
                                             start=True, stop=True)
                            if e >= 6 - NEV_DVE:
                                nc.vector.tensor_copy(awf[:, base:base + 1024],
                                                      psb[:])
                            else:
                                nc.scalar.copy(awf[:, base:base + 1024], psb[:])
                        c4 = c4p.tile([P, 4, CW], f16, tag="c4")
                        for h, (gTh, gBh) in enumerate(ghs):
                            hs = slice(h * GW, (h + 1) * GW)
                            nc.vector.tensor_tensor(c4[:, 0:2, hs], gTh[:],
                                                    aw[:, 0:2, hs], Alu.mult)
                            nc.vector.tensor_tensor(c4[:, 2:4, hs], gBh[:],
                                                    aw[:, 2:4, hs], Alu.mult)
                        c4_prev.append((k, c4))
                        # GEMM runs one tap behind so PE never stalls on the
                        # premult: by the time this tap's matmuls issue, the
                        # previous tap's c4 is already finished on DVE.
                        if len(c4_prev) > 1 or k == K - 1:
                            for kg, c4g in list(c4_prev if k == K - 1
                                                else c4_prev[:1]):
                                for j in range(CW // SUB):
                                    for si in range(4):
                                        nc.tensor.matmul(
                                            out_ps[:, j * SUB:(j + 1) * SUB],
                                            WkT[:, kg * P:(kg + 1) * P],
                                            c4g[:, si, j * SUB:(j + 1) * SUB],
                                            start=(kg == 0 and si == 0),
                                            stop=(kg == K - 1 and si == 3),
                                            skip_group_check=True)
                                c4_prev.pop(0)
                    osb = osp.tile([P, CW], f32, tag="osb")
                    if c % 2 == 0:
                        nc.scalar.copy(osb[:], out_ps[:])
                    else:
                        nc.vector.tensor_copy(osb[:], out_ps[:])
                    nc.sync.dma_start(out[:, c * CW:(c + 1) * CW], osb[:])
    nc.compile()
    return nc


_NC = None


def kernel(x, offset, weight):
    global _NC
    if _NC is None:
        _NC = build_nc()
    from concourse.bass_utils import run_bass_kernel_spmd
    B = x.shape[0]
    w2 = np.ascontiguousarray(weight.reshape(P, 1152)).astype(np.float32)
    in_maps = []
    for b in range(B):
        in_maps.append({
            "x": np.ascontiguousarray(np.asarray(x)[b].reshape(P, NPOS), dtype=np.float32),
            "offset": np.ascontiguousarray(np.asarray(offset)[b].reshape(18, NPOS), dtype=np.float32),
            "weight": w2,
        })
    res = run_bass_kernel_spmd(_NC, in_maps, list(range(B)))
    outs = [res.results[b]["out"].reshape(P, H, W) for b in range(B)]
    return np.stack(outs).astype(np.float32)


# revision 38
# speedup vs baseline: 1.4679x; 1.0102x over previous
"""DeformConv2d forward on 8 Trainium2 NeuronCores (Bass/Tile).

x[8,128,96,96] f32, offset[8,18,96,96] f32, weight[128,128,3,3] f32
-> out[8,128,96,96] f32. Deformable 3x3 conv, pad 1, stride 1, bilinear
sampling with zero padding. Data-parallel over batch: one element per core.

v3: overlapped prep (offsets loaded first; index/weight math split across
DVE and GPSIMD; per-tap idxw loads so gathers start as soon as x_t and the
first tap's indices are ready), swizzled idx evac for contiguous wrap
loads, ACT/Pool/DVE-balanced PSUM evacuation, direct fp16 premultiply.
"""
import sys
if '/opt/trn_rl_repo' not in sys.path:
    sys.path.insert(0, '/opt/trn_rl_repo')

import os
import numpy as np

import concourse.bacc as bacc_mod
import concourse.mybir as mybir
import concourse.tile as tile
from concourse.ap import AP

f32 = mybir.dt.float32
f16 = mybir.dt.float16
i16 = mybir.dt.int16
i32 = mybir.dt.int32
Alu = mybir.AluOpType

P = 128
H = W = 96
NPOS = H * W              # 9216
NT = NPOS // P            # 72 position tiles
K = 9
NROW = NPOS - 1           # pair windows in x_t
CW = 1536                 # main-loop position chunk
NCH = NPOS // CW          # 6 chunks
SUB = int(os.environ.get("SUB", "512"))  # GEMM moving sub-chunk
NEV_DVE = int(os.environ.get("NEV_DVE", "1"))   # aw evacs on DVE (of 6)
NEV_POOL = int(os.environ.get("NEV_POOL", "0"))  # aw evacs on Pool (of 6)
NFUSE = int(os.environ.get("NFUSE", "2"))   # psb chunks premultiplied from PSUM


def _h(ap_or_handle):
    return ap_or_handle.tensor if hasattr(ap_or_handle, 'tensor') else ap_or_handle


def build_nc():
    nc = bacc_mod.Bacc()
    x_in = nc.declare_dram_parameter("x", [P, NPOS], f32, isOutput=False)
    off_in = nc.declare_dram_parameter("offset", [18, NPOS], f32, isOutput=False)
    w_in = nc.declare_dram_parameter("weight", [P, 1152], f32, isOutput=False)
    out = nc.declare_dram_parameter("out", [P, NPOS], f32, isOutput=True)

    with tile.TileContext(nc) as tc:
        with tc.tile_pool(name="const", bufs=1) as cpool, \
             tc.tile_pool(name="persist", bufs=1) as ppool, \
             tc.tile_pool(name="dram", bufs=1, space="DRAM") as dpool:
            x_t = dpool.tile([NPOS, P], f16, name="x_t")
            w_rows = dpool.tile([36, NPOS], f16, name="w_rows")
            idx_d = dpool.tile([16, 18 * 576], i16, name="idx_d")
            # ---------- constants ----------
            ident16 = cpool.tile([P, P], f16)
            ident32 = cpool.tile([P, P], f32)
            ones_row = cpool.tile([1, P], f16)
            nc.vector.memset(ones_row[:], 1.0)
            onesP = cpool.tile([P, P], f32)
            nc.vector.memset(onesP[:], 1.0)
            ramp128 = cpool.tile([P, P], f32)
            nc.vector.tensor_tensor_scan(ramp128[:], onesP[:], onesP[:], -1.0,
                                         Alu.mult, Alu.add)
            pcol_d = dpool.tile([1, P], f32, name="pcol_d")
            nc.sync.dma_start(pcol_d[:], ramp128[0:1, :])
            pcol = cpool.tile([P, 1], f32)
            src_p = AP(tensor=_h(pcol_d), offset=0, ap=[[1, P], [1, 1]])
            nc.sync.dma_start(pcol[:], src_p)
            nc.vector.tensor_scalar(ident32[:], ramp128[:], pcol[:], None,
                                    Alu.is_equal)
            nc.vector.tensor_copy(ident16[:], ident32[:])

            # ---------- persistent tiles ----------
            # three idx groups: tap 0 alone (unblocks the first gather
            # early), taps 1-3, taps 4-8 (replicated while the loop runs)
            IDX_GROUPS = [(0, 1), (1, 4), (4, 9)]
            idxw_g = [ppool.tile([P, 1152 * (b - a)], i16, name=f"idxw_g{a}")
                      for a, b in IDX_GROUPS]
            WkT = ppool.tile([P, K * P], f16)

            with tc.tile_pool(name="prepA", bufs=2) as pa, \
                 tc.tile_pool(name="prepEv", bufs=2) as pev, \
                 tc.tile_pool(name="prepAp", bufs=2, space="PSUM") as pap, \
                 tc.tile_pool(name="prepB", bufs=1) as pb, \
                 tc.tile_pool(name="prepBp", bufs=2, space="PSUM") as pbp:
                # offsets + weights first: unblocks DVE/GPSIMD math and WkT
                # while the larger x load streams in behind them.
                off_sb = pb.tile([18, NPOS], f32, tag="offsb")
                nc.sync.dma_start(off_sb[:], off_in[:])
                w_sb = pa.tile([P, 1152], f32, tag="wsb")
                nc.sync.dma_start(w_sb[:], w_in[:])

                # ---------- phase B: offsets -> position-packed ----------
                offt = pb.tile([P, NT * 18], f32, tag="offt")
                for tg in range(3):
                    pso = pbp.tile([P, 24 * 18], f32, tag="pso")
                    for j in range(24):
                        t = tg * 24 + j
                        nc.tensor.transpose(pso[:, j * 18:(j + 1) * 18],
                                            off_sb[0:18, t * P:(t + 1) * P],
                                            ident32[0:18, 0:18])
                    nc.vector.tensor_copy(offt[:, tg * 432:(tg + 1) * 432], pso[:])

                # ---------- phase A: x -> x_t, weights -> WkT ----------
                x16 = pb.tile([P, NPOS], f16, tag="x16")
                for xs in range(4):
                    x_sl = pa.tile([P, NPOS // 4], f32, tag="xsl")
                    nc.sync.dma_start(
                        x_sl[:], x_in[:, xs * (NPOS // 4):(xs + 1) * (NPOS // 4)])
                    nc.scalar.copy(
                        x16[:, xs * (NPOS // 4):(xs + 1) * (NPOS // 4)], x_sl[:])
                for tg in range(3):
                    ev24 = pev.tile([P, 24 * P], f16, tag="ev24")
                    for q in range(3):
                        tq = tg * 3 + q
                        pt8 = pap.tile([P, 8 * P], f16, tag="pt8")
                        for j in range(8):
                            t = tq * 8 + j
                            nc.tensor.transpose(pt8[:, j * P:(j + 1) * P],
                                                x16[:, t * P:(t + 1) * P],
                                                ident16[:])
                        nc.scalar.copy(ev24[:, q * 1024:(q + 1) * 1024], pt8[:])
                    dst = AP(tensor=_h(x_t), offset=tg * 3072 * P,
                             ap=[[P, P], [128 * P, 24], [1, P]])
                    nc.sync.dma_start(dst,
                                      ev24[:].rearrange("r (j c) -> r j c", j=24))

                w16 = pa.tile([P, 1152], f16, tag="w16")
                nc.scalar.copy(w16[:], w_sb[:])
                for k in range(K):
                    wkc = pa.tile([P, P], f16, tag="wkc")
                    nc.vector.tensor_copy(wkc[:], w16[:, k:1152:9])
                    ptw = pap.tile([P, 8 * P], f16, tag="pt8")
                    nc.tensor.transpose(ptw[:, 0:P], wkc[:], ident16[:])
                    nc.scalar.copy(WkT[:, k * P:(k + 1) * P], ptw[:, 0:P])

                # ---------- phase C: math (y-chain on DVE, x-chain on Pool) --
                NF = K * NT  # 648

                def mt(tag, dt=f32):
                    return pb.tile([P, NF], dt, tag=tag, name=tag)

                posf = pb.tile([P, NT], f32, tag="posf")
                nc.vector.tensor_scalar(posf[:], ramp128[:, 0:NT], 128.0, None,
                                        Alu.mult)
                nc.vector.tensor_scalar(posf[:], posf[:], pcol[:], None, Alu.add)

                q0i = pb.tile([P, NT], i32, tag="q0i")
                tmpq = pb.tile([P, NT], f32, tag="tmpq")
                nc.vector.tensor_scalar(tmpq[:], posf[:], 1.0 / 96.0, None, Alu.mult)
                nc.vector.tensor_copy(q0i[:], tmpq[:])
                q0 = pb.tile([P, NT], f32, tag="q0")
                nc.vector.tensor_copy(q0[:], q0i[:])
                r0 = pb.tile([P, NT], f32, tag="r0")
                nc.vector.scalar_tensor_tensor(r0[:], q0[:], -96.0, posf[:],
                                               Alu.mult, Alu.add)
                ltz = pb.tile([P, NT], f32, tag="ltz")
                nc.vector.tensor_scalar(ltz[:], r0[:], 0.0, None, Alu.is_lt)
                gez = pb.tile([P, NT], f32, tag="gez")
                nc.vector.tensor_scalar(gez[:], r0[:], 96.0, None, Alu.is_ge)
                Rr = pb.tile([P, NT], f32, tag="Rr")
                nc.vector.tensor_tensor(Rr[:], q0[:], ltz[:], Alu.subtract)
                nc.vector.tensor_tensor(Rr[:], Rr[:], gez[:], Alu.add)
                Cc = pb.tile([P, NT], f32, tag="Cc")
                nc.vector.scalar_tensor_tensor(Cc[:], ltz[:], 96.0, r0[:],
                                               Alu.mult, Alu.add)
                nc.vector.scalar_tensor_tensor(Cc[:], gez[:], -96.0, Cc[:],
                                               Alu.mult, Alu.add)

                BY = mt("BY", f16)
                BX = mt("BX", f16)
                for k in range(K):
                    ky, kx = k // 3, k % 3
                    nc.vector.tensor_scalar(BY[:, k * NT:(k + 1) * NT], Rr[:],
                                            float(ky - 1), None, Alu.add)
                    nc.gpsimd.tensor_scalar(BX[:, k * NT:(k + 1) * NT], Cc[:],
                                            float(kx - 1), None, Alu.add)

                offv = offt[:].rearrange("p (t pl) -> p pl t", pl=18)
                py = mt("py")
                px = mt("px")
                nc.vector.tensor_tensor(
                    py[:].rearrange("p (k t) -> p k t", k=K),
                    offv[:, 0:18:2, :],
                    BY[:].rearrange("p (k t) -> p k t", k=K), Alu.add)
                nc.gpsimd.tensor_tensor(
                    px[:].rearrange("p (k t) -> p k t", k=K),
                    offv[:, 1:18:2, :],
                    BX[:].rearrange("p (k t) -> p k t", k=K), Alu.add)

                def floor_frac(eng, v, pfx):
                    vi = mt(pfx + "i", i16)
                    eng.tensor_copy(vi[:], v[:])
                    vf = mt(pfx + "f")
                    eng.tensor_copy(vf[:], vi[:])
                    fr = mt(pfx + "fr")
                    eng.tensor_tensor(fr[:], v[:], vf[:], Alu.subtract)
                    ng = mt(pfx + "ng")
                    eng.tensor_scalar(ng[:], fr[:], 0.0, None, Alu.is_lt)
                    eng.tensor_tensor(vf[:], vf[:], ng[:], Alu.subtract)
                    eng.tensor_tensor(fr[:], fr[:], ng[:], Alu.add)
                    return vf, fr

                y0, fy = floor_frac(nc.vector, py, "y")
                x0, fx = floor_frac(nc.gpsimd, px, "x")

                def range_mask(eng, v, lo, hi, pfx):
                    g = mt(pfx + "g")
                    eng.tensor_scalar(g[:], v[:], float(lo), None, Alu.is_ge)
                    l = mt(pfx + "l")
                    eng.tensor_scalar(l[:], v[:], float(hi), None, Alu.is_le)
                    eng.tensor_tensor(g[:], g[:], l[:], Alu.mult)
                    return g

                vt = range_mask(nc.vector, y0, 0, 95, "vt")
                vb = range_mask(nc.vector, y0, -1, 94, "vb")
                inr = range_mask(nc.gpsimd, x0, 0, 94, "inr")
                omfy = mt("omfy", f16)
                nc.vector.tensor_scalar(omfy[:], fy[:], -1.0, 1.0, Alu.mult, Alu.add)
                omfx = mt("omfx", f16)
                nc.gpsimd.tensor_scalar(omfx[:], fx[:], -1.0, 1.0, Alu.mult, Alu.add)
                wtop = mt("wtop", f16)
                nc.vector.tensor_tensor(wtop[:], omfy[:], vt[:], Alu.mult)
                wbot = mt("wbot", f16)
                nc.vector.tensor_tensor(wbot[:], fy[:], vb[:], Alu.mult)
                em1 = mt("em1")
                nc.vector.tensor_scalar(em1[:], x0[:], -1.0, None, Alu.is_equal)
                e95 = mt("e95")
                nc.vector.tensor_scalar(e95[:], x0[:], 95.0, None, Alu.is_equal)
                s0 = mt("s0", f16)
                s1 = mt("s1", f16)
                tmp = mt("tmp", f16)
                nc.gpsimd.tensor_tensor(s0[:], inr[:], omfx[:], Alu.mult)
                nc.gpsimd.tensor_tensor(tmp[:], em1[:], fx[:], Alu.mult)
                nc.gpsimd.tensor_tensor(s0[:], s0[:], tmp[:], Alu.add)
                nc.gpsimd.tensor_tensor(s1[:], inr[:], fx[:], Alu.mult)
                nc.gpsimd.tensor_tensor(tmp[:], e95[:], omfx[:], Alu.mult)
                nc.gpsimd.tensor_tensor(s1[:], s1[:], tmp[:], Alu.add)

                A0 = mt("A0", f16)
                A1 = mt("A1", f16)
                B0 = mt("B0", f16)
                B1 = mt("B1", f16)
                nc.vector.tensor_tensor(A0[:], wtop[:], s0[:], Alu.mult)
                nc.vector.tensor_tensor(A1[:], wtop[:], s1[:], Alu.mult)
                nc.vector.tensor_tensor(B0[:], wbot[:], s0[:], Alu.mult)
                nc.vector.tensor_tensor(B1[:], wbot[:], s1[:], Alu.mult)

                x0c = mt("x0c")
                nc.vector.tensor_scalar(x0c[:], x0[:], 0.0, 94.0, Alu.max, Alu.min)
                y0c = mt("y0c")
                nc.vector.tensor_scalar(y0c[:], y0[:], 0.0, 95.0, Alu.max, Alu.min)
                y1p = mt("y1p")
                nc.vector.tensor_scalar(y1p[:], y0[:], -1.0, 94.0, Alu.max, Alu.min)
                x0c96 = mt("x0c96")
                nc.vector.tensor_scalar(x0c96[:], x0c[:], 96.0, None, Alu.add)
                IDXT = mt("IDXT")
                nc.vector.scalar_tensor_tensor(IDXT[:], y0c[:], 96.0, x0c[:],
                                               Alu.mult, Alu.add)
                IDXB = mt("IDXB")
                nc.vector.scalar_tensor_tensor(IDXB[:], y1p[:], 96.0, x0c96[:],
                                               Alu.mult, Alu.add)

                # ---------- phase D + per-tap idxw load ----------
                for k in range(K):
                    psw = pbp.tile([NT, 4 * P], f16, tag="psw")
                    for s, tt_ in enumerate((A0, A1, B0, B1)):
                        nc.tensor.transpose(psw[:, s * P:(s + 1) * P],
                                            tt_[:, k * NT:(k + 1) * NT],
                                            ident16[:])
                    evw = pb.tile([NT, 4 * P], f16, tag="evw")
                    nc.scalar.copy(evw[:], psw[:])
                    dstw = AP(tensor=_h(w_rows), offset=(4 * k) * NPOS,
                              ap=[[P, NT], [NPOS, 4], [1, P]])
                    nc.scalar.dma_start(dstw,
                                        evw[:].rearrange("c (s e) -> c s e",
                                                         s=4))

                    psi = pbp.tile([NT, 2 * P], f32, tag="psi")
                    nc.tensor.transpose(psi[:, 0:P],
                                        IDXT[:, k * NT:(k + 1) * NT], ident32[:])
                    nc.tensor.transpose(psi[:, P:2 * P],
                                        IDXB[:, k * NT:(k + 1) * NT], ident32[:])
                    # 16-wrap swizzle on evac: evi[t, h*128 + a*8 + b] =
                    # psi[t, h*128 + b*16 + a]  (a = pix%16, b = (pix%128)//16)
                    evi = pb.tile([NT, 2 * P], i16, tag="evi")
                    nc.vector.tensor_copy(
                        evi[:].rearrange("t (h a b) -> t h a b", h=2, a=16),
                        psi[:].rearrange("t (h b a) -> t h a b", h=2, b=8))
                    # idx_d[a][2k+h][t*8+b] — contiguous wrap-load layout
                    for hh in range(2):
                        dsti = AP(tensor=_h(idx_d), offset=(2 * k + hh) * 576,
                                  ap=[[8, NT], [18 * 576, 16], [1, 8]])
                        nc.sync.dma_start(
                            dsti,
                            evi[:, hh * P:(hh + 1) * P].rearrange(
                                "t (a b) -> t a b", a=16))
                    # wrap-load + replicate once per idx group
                    for gi, (ga, gb) in enumerate(IDX_GROUPS):
                        if k != gb - 1:
                            continue
                        ixg = idxw_g[gi]
                        srcw = AP(tensor=_h(idx_d), offset=(2 * ga) * 576,
                                  ap=[[18 * 576, 16], [1, 1152 * (gb - ga)]])
                        nc.scalar.dma_start(ixg[0:16, :], srcw)
                        nc.scalar.dma_start(ixg[16:32, :], ixg[0:16, :])
                        nc.scalar.dma_start(ixg[32:64, :], ixg[0:32, :])
                        nc.scalar.dma_start(ixg[64:128, :], ixg[0:64, :])

            # ---------- phase F: main loop ----------
            xt_win = AP(tensor=_h(x_t), offset=0, ap=[[P, NROW], [1, 2 * P]])
            with tc.tile_pool(name="g", bufs=4) as gp, \
                 tc.tile_pool(name="aw", bufs=2) as awp, \
                 tc.tile_pool(name="c4", bufs=3) as c4p, \
                 tc.tile_pool(name="wstp", bufs=2) as wsp, \
                 tc.tile_pool(name="ops", bufs=2) as osp, \
                 tc.tile_pool(name="awps", bufs=2, space="PSUM") as awps, \
                 tc.tile_pool(name="outps", bufs=1, space="PSUM") as outps:
                for c in range(NCH):
                    out_ps = outps.tile([P, CW], f32, tag="ops", name="out_ps")
                    c4_prev = []
                    for k in range(K):
                        # DGE inflight ring caps a transpose-gather at
                        # num_idxs*2/16+2 < 128 descriptors per DMA engine,
                        # i.e. num_idxs <= 896 — gather in GW=768 pieces.
                        GW = 768
                        ghs = []
                        for h in range(CW // GW):
                            gTh = gp.tile([P, 2, GW], f16, tag="gT", name="gTh")
                            gBh = gp.tile([P, 2, GW], f16, tag="gB", name="gBh")
                            gi = 0 if k == 0 else (1 if k < 4 else 2)
                            ka = k - IDX_GROUPS[gi][0]
                            i0t = ka * 1152 + (c * CW + h * GW) // 16
                            i0b = ka * 1152 + 576 + (c * CW + h * GW) // 16
                            nc.gpsimd.dma_gather(
                                gTh[:], xt_win,
                                idxw_g[gi][:, i0t:i0t + GW // 16],
                                num_idxs=GW, num_idxs_reg=GW,
                                elem_size=2 * P, elem_step=P, transpose=True)
                            nc.gpsimd.dma_gather(
                                gBh[:], xt_win,
                                idxw_g[gi][:, i0b:i0b + GW // 16],
                                num_idxs=GW, num_idxs_reg=GW,
                                elem_size=2 * P, elem_step=P, transpose=True)
                            ghs.append((gTh, gBh))
                        wst = wsp.tile([1, 4 * CW], f16, tag="wst")
                        wsrc = AP(tensor=_h(w_rows),
                                  offset=(4 * k) * NPOS + c * CW,
                                  ap=[[NPOS, 4], [1, CW]])
                        nc.sync.dma_start(
                            wst[:].rearrange("p (s e) -> p s e", s=4),
                            wsrc.unsqueeze(0))
                        aw = awp.tile([P, 4, CW], f16, tag="aw")
                        awf = aw[:].rearrange("p s c -> p (s c)")
                        for e in range(6):
                            psb = awps.tile([P, 1024], f32, tag="psb", name="psb")
                            base = e * 1024
                            nc.tensor.matmul(psb[:, 0:SUB], ones_row[:],
                                             wst[0:1, base:base + SUB],
                                             start=True, stop=True)
                            nc.tensor.matmul(psb[:, SUB:1024], ones_row[:],
                                             wst[0:1, base + SUB:base + 1024],
                                             start=True, stop=True)
                            if e >= 6 - NEV_DVE:
                                nc.vector.tensor_copy(awf[:, base:base + 1024],
                                                      psb[:])
                            else:
                                nc.scalar.copy(awf[:, base:base + 1024], psb[:])
                        c4 = c4p.tile([P, 4, CW], f16, tag="c4")
                        for h, (gTh, gBh) in enumerate(ghs):
                            hs = slice(h * GW, (h + 1) * GW)
                            nc.vector.tensor_tensor(c4[:, 0:2, hs], gTh[:],
                                                    aw[:, 0:2, hs], Alu.mult)
                            nc.vector.tensor_tensor(c4[:, 2:4, hs], gBh[:],
                                                    aw[:, 2:4, hs], Alu.mult)
                        c4_prev.append((k, c4))
                        # GEMM runs one tap behind so PE never stalls on the
                        # premult: by the time this tap's matmuls issue, the
                        # previous tap's c4 is already finished on DVE.
                        if len(c4_prev) > 1 or k == K - 1:
                            for kg, c4g in list(c4_prev if k == K - 1
                                                else c4_prev[:1]):
                                for j in range(CW // SUB):
                                    for si in range(4):
                                        nc.tensor.matmul(
                                            out_ps[:, j * SUB:(j + 1) * SUB],
                                            WkT[:, kg * P:(kg + 1) * P],
                                            c4g[:, si, j * SUB:(j + 1) * SUB],
                                            start=(kg == 0 and si == 0),
                                            stop=(kg == K - 1 and si == 3),
                                            skip_group_check=True)
                                c4_prev.pop(0)
                    osb = osp.tile([P, CW], f32, tag="osb")
                    if c % 2 == 0:
                        nc.scalar.copy(osb[:], out_ps[:])
                    else:
                        nc.vector.tensor_copy(osb[:], out_ps[:])
                    nc.scalar.dma_start(out[:, c * CW:(c + 1) * CW], osb[:])
    nc.compile()
    return nc


_NC = None


def kernel(x, offset, weight):
    global _NC
    if _NC is None:
        _NC = build_nc()
    from concourse.bass_utils import run_bass_kernel_spmd
    B = x.shape[0]
    w2 = np.ascontiguousarray(weight.reshape(P, 1152)).astype(np.float32)
    in_maps = []
    for b in range(B):
        in_maps.append({
            "x": np.ascontiguousarray(np.asarray(x)[b].reshape(P, NPOS), dtype=np.float32),
            "offset": np.ascontiguousarray(np.asarray(offset)[b].reshape(18, NPOS), dtype=np.float32),
            "weight": w2,
        })
    res = run_bass_kernel_spmd(_NC, in_maps, list(range(B)))
    outs = [res.results[b]["out"].reshape(P, H, W) for b in range(B)]
    return np.stack(outs).astype(np.float32)


# revision 39
# speedup vs baseline: 1.4749x; 1.0048x over previous
"""DeformConv2d forward on 8 Trainium2 NeuronCores (Bass/Tile).

x[8,128,96,96] f32, offset[8,18,96,96] f32, weight[128,128,3,3] f32
-> out[8,128,96,96] f32. Deformable 3x3 conv, pad 1, stride 1, bilinear
sampling with zero padding. Data-parallel over batch: one element per core.

v3: overlapped prep (offsets loaded first; index/weight math split across
DVE and GPSIMD; per-tap idxw loads so gathers start as soon as x_t and the
first tap's indices are ready), swizzled idx evac for contiguous wrap
loads, ACT/Pool/DVE-balanced PSUM evacuation, direct fp16 premultiply.
"""
import sys
if '/opt/trn_rl_repo' not in sys.path:
    sys.path.insert(0, '/opt/trn_rl_repo')

import os
import numpy as np

import concourse.bacc as bacc_mod
import concourse.mybir as mybir
import concourse.tile as tile
from concourse.ap import AP

f32 = mybir.dt.float32
f16 = mybir.dt.float16
i16 = mybir.dt.int16
i32 = mybir.dt.int32
Alu = mybir.AluOpType

P = 128
H = W = 96
NPOS = H * W              # 9216
NT = NPOS // P            # 72 position tiles
K = 9
NROW = NPOS - 1           # pair windows in x_t
CW = 1536                 # main-loop position chunk
NCH = NPOS // CW          # 6 chunks
SUB = int(os.environ.get("SUB", "512"))  # GEMM moving sub-chunk
NEV_DVE = int(os.environ.get("NEV_DVE", "1"))   # aw evacs on DVE (of 6)
NEV_POOL = int(os.environ.get("NEV_POOL", "0"))  # aw evacs on Pool (of 6)
NFUSE = int(os.environ.get("NFUSE", "2"))   # psb chunks premultiplied from PSUM


def _h(ap_or_handle):
    return ap_or_handle.tensor if hasattr(ap_or_handle, 'tensor') else ap_or_handle


def build_nc():
    nc = bacc_mod.Bacc()
    x_in = nc.declare_dram_parameter("x", [P, NPOS], f32, isOutput=False)
    off_in = nc.declare_dram_parameter("offset", [18, NPOS], f32, isOutput=False)
    w_in = nc.declare_dram_parameter("weight", [P, 1152], f32, isOutput=False)
    out = nc.declare_dram_parameter("out", [P, NPOS], f32, isOutput=True)

    with tile.TileContext(nc) as tc:
        with tc.tile_pool(name="const", bufs=1) as cpool, \
             tc.tile_pool(name="persist", bufs=1) as ppool, \
             tc.tile_pool(name="dram", bufs=1, space="DRAM") as dpool:
            x_t = dpool.tile([NPOS, P], f16, name="x_t")
            w_rows = dpool.tile([36, NPOS], f16, name="w_rows")
            idx_d = dpool.tile([16, 18 * 576], i16, name="idx_d")
            # ---------- constants ----------
            ident16 = cpool.tile([P, P], f16)
            ident32 = cpool.tile([P, P], f32)
            ones_row = cpool.tile([1, P], f16)
            nc.vector.memset(ones_row[:], 1.0)
            onesP = cpool.tile([P, P], f32)
            nc.vector.memset(onesP[:], 1.0)
            ramp128 = cpool.tile([P, P], f32)
            nc.vector.tensor_tensor_scan(ramp128[:], onesP[:], onesP[:], -1.0,
                                         Alu.mult, Alu.add)
            pcol_d = dpool.tile([1, P], f32, name="pcol_d")
            nc.sync.dma_start(pcol_d[:], ramp128[0:1, :])
            pcol = cpool.tile([P, 1], f32)
            src_p = AP(tensor=_h(pcol_d), offset=0, ap=[[1, P], [1, 1]])
            nc.sync.dma_start(pcol[:], src_p)
            nc.vector.tensor_scalar(ident32[:], ramp128[:], pcol[:], None,
                                    Alu.is_equal)
            nc.vector.tensor_copy(ident16[:], ident32[:])

            # ---------- persistent tiles ----------
            # three idx groups: tap 0 alone (unblocks the first gather
            # early), taps 1-3, taps 4-8 (replicated while the loop runs)
            IDX_GROUPS = [(0, 1), (1, 4), (4, 9)]
            idxw_g = [ppool.tile([P, 1152 * (b - a)], i16, name=f"idxw_g{a}")
                      for a, b in IDX_GROUPS]
            WkT = ppool.tile([P, K * P], f16)

            with tc.tile_pool(name="prepA", bufs=2) as pa, \
                 tc.tile_pool(name="prepEv", bufs=2) as pev, \
                 tc.tile_pool(name="prepAp", bufs=2, space="PSUM") as pap, \
                 tc.tile_pool(name="prepB", bufs=1) as pb, \
                 tc.tile_pool(name="prepBp", bufs=2, space="PSUM") as pbp:
                # offsets + weights first: unblocks DVE/GPSIMD math and WkT
                # while the larger x load streams in behind them.
                off_sb = pb.tile([18, NPOS], f32, tag="offsb")
                nc.scalar.dma_start(off_sb[:], off_in[:])
                w_sb = pa.tile([P, 1152], f32, tag="wsb")
                nc.sync.dma_start(w_sb[:], w_in[:])

                # ---------- phase B: offsets -> position-packed ----------
                offt = pb.tile([P, NT * 18], f32, tag="offt")
                for tg in range(3):
                    pso = pbp.tile([P, 24 * 18], f32, tag="pso")
                    for j in range(24):
                        t = tg * 24 + j
                        nc.tensor.transpose(pso[:, j * 18:(j + 1) * 18],
                                            off_sb[0:18, t * P:(t + 1) * P],
                                            ident32[0:18, 0:18])
                    nc.vector.tensor_copy(offt[:, tg * 432:(tg + 1) * 432], pso[:])

                # ---------- phase A: x -> x_t, weights -> WkT ----------
                x16 = pb.tile([P, NPOS], f16, tag="x16")
                for xs in range(4):
                    x_sl = pa.tile([P, NPOS // 4], f32, tag="xsl")
                    nc.sync.dma_start(
                        x_sl[:], x_in[:, xs * (NPOS // 4):(xs + 1) * (NPOS // 4)])
                    nc.scalar.copy(
                        x16[:, xs * (NPOS // 4):(xs + 1) * (NPOS // 4)], x_sl[:])
                for tg in range(3):
                    ev24 = pev.tile([P, 24 * P], f16, tag="ev24")
                    for q in range(3):
                        tq = tg * 3 + q
                        pt8 = pap.tile([P, 8 * P], f16, tag="pt8")
                        for j in range(8):
                            t = tq * 8 + j
                            nc.tensor.transpose(pt8[:, j * P:(j + 1) * P],
                                                x16[:, t * P:(t + 1) * P],
                                                ident16[:])
                        nc.scalar.copy(ev24[:, q * 1024:(q + 1) * 1024], pt8[:])
                    dst = AP(tensor=_h(x_t), offset=tg * 3072 * P,
                             ap=[[P, P], [128 * P, 24], [1, P]])
                    nc.sync.dma_start(dst,
                                      ev24[:].rearrange("r (j c) -> r j c", j=24))

                w16 = pa.tile([P, 1152], f16, tag="w16")
                nc.scalar.copy(w16[:], w_sb[:])
                for k in range(K):
                    wkc = pa.tile([P, P], f16, tag="wkc")
                    nc.vector.tensor_copy(wkc[:], w16[:, k:1152:9])
                    ptw = pap.tile([P, 8 * P], f16, tag="pt8")
                    nc.tensor.transpose(ptw[:, 0:P], wkc[:], ident16[:])
                    nc.scalar.copy(WkT[:, k * P:(k + 1) * P], ptw[:, 0:P])

                # ---------- phase C: math (y-chain on DVE, x-chain on Pool) --
                NF = K * NT  # 648

                def mt(tag, dt=f32):
                    return pb.tile([P, NF], dt, tag=tag, name=tag)

                posf = pb.tile([P, NT], f32, tag="posf")
                nc.vector.tensor_scalar(posf[:], ramp128[:, 0:NT], 128.0, None,
                                        Alu.mult)
                nc.vector.tensor_scalar(posf[:], posf[:], pcol[:], None, Alu.add)

                q0i = pb.tile([P, NT], i32, tag="q0i")
                tmpq = pb.tile([P, NT], f32, tag="tmpq")
                nc.vector.tensor_scalar(tmpq[:], posf[:], 1.0 / 96.0, None, Alu.mult)
                nc.vector.tensor_copy(q0i[:], tmpq[:])
                q0 = pb.tile([P, NT], f32, tag="q0")
                nc.vector.tensor_copy(q0[:], q0i[:])
                r0 = pb.tile([P, NT], f32, tag="r0")
                nc.vector.scalar_tensor_tensor(r0[:], q0[:], -96.0, posf[:],
                                               Alu.mult, Alu.add)
                ltz = pb.tile([P, NT], f32, tag="ltz")
                nc.vector.tensor_scalar(ltz[:], r0[:], 0.0, None, Alu.is_lt)
                gez = pb.tile([P, NT], f32, tag="gez")
                nc.vector.tensor_scalar(gez[:], r0[:], 96.0, None, Alu.is_ge)
                Rr = pb.tile([P, NT], f32, tag="Rr")
                nc.vector.tensor_tensor(Rr[:], q0[:], ltz[:], Alu.subtract)
                nc.vector.tensor_tensor(Rr[:], Rr[:], gez[:], Alu.add)
                Cc = pb.tile([P, NT], f32, tag="Cc")
                nc.vector.scalar_tensor_tensor(Cc[:], ltz[:], 96.0, r0[:],
                                               Alu.mult, Alu.add)
                nc.vector.scalar_tensor_tensor(Cc[:], gez[:], -96.0, Cc[:],
                                               Alu.mult, Alu.add)

                BY = mt("BY", f16)
                BX = mt("BX", f16)
                for k in range(K):
                    ky, kx = k // 3, k % 3
                    nc.vector.tensor_scalar(BY[:, k * NT:(k + 1) * NT], Rr[:],
                                            float(ky - 1), None, Alu.add)
                    nc.gpsimd.tensor_scalar(BX[:, k * NT:(k + 1) * NT], Cc[:],
                                            float(kx - 1), None, Alu.add)

                offv = offt[:].rearrange("p (t pl) -> p pl t", pl=18)
                py = mt("py")
                px = mt("px")
                nc.vector.tensor_tensor(
                    py[:].rearrange("p (k t) -> p k t", k=K),
                    offv[:, 0:18:2, :],
                    BY[:].rearrange("p (k t) -> p k t", k=K), Alu.add)
                nc.gpsimd.tensor_tensor(
                    px[:].rearrange("p (k t) -> p k t", k=K),
                    offv[:, 1:18:2, :],
                    BX[:].rearrange("p (k t) -> p k t", k=K), Alu.add)

                def floor_frac(eng, v, pfx):
                    vi = mt(pfx + "i", i16)
                    eng.tensor_copy(vi[:], v[:])
                    vf = mt(pfx + "f")
                    eng.tensor_copy(vf[:], vi[:])
                    fr = mt(pfx + "fr")
                    eng.tensor_tensor(fr[:], v[:], vf[:], Alu.subtract)
                    ng = mt(pfx + "ng")
                    eng.tensor_scalar(ng[:], fr[:], 0.0, None, Alu.is_lt)
                    eng.tensor_tensor(vf[:], vf[:], ng[:], Alu.subtract)
                    eng.tensor_tensor(fr[:], fr[:], ng[:], Alu.add)
                    return vf, fr

                y0, fy = floor_frac(nc.vector, py, "y")
                x0, fx = floor_frac(nc.gpsimd, px, "x")

                def range_mask(eng, v, lo, hi, pfx):
                    g = mt(pfx + "g")
                    eng.tensor_scalar(g[:], v[:], float(lo), None, Alu.is_ge)
                    l = mt(pfx + "l")
                    eng.tensor_scalar(l[:], v[:], float(hi), None, Alu.is_le)
                    eng.tensor_tensor(g[:], g[:], l[:], Alu.mult)
                    return g

                vt = range_mask(nc.vector, y0, 0, 95, "vt")
                vb = range_mask(nc.vector, y0, -1, 94, "vb")
                inr = range_mask(nc.gpsimd, x0, 0, 94, "inr")
                omfy = mt("omfy", f16)
                nc.vector.tensor_scalar(omfy[:], fy[:], -1.0, 1.0, Alu.mult, Alu.add)
                omfx = mt("omfx", f16)
                nc.gpsimd.tensor_scalar(omfx[:], fx[:], -1.0, 1.0, Alu.mult, Alu.add)
                wtop = mt("wtop", f16)
                nc.vector.tensor_tensor(wtop[:], omfy[:], vt[:], Alu.mult)
                wbot = mt("wbot", f16)
                nc.vector.tensor_tensor(wbot[:], fy[:], vb[:], Alu.mult)
                em1 = mt("em1")
                nc.vector.tensor_scalar(em1[:], x0[:], -1.0, None, Alu.is_equal)
                e95 = mt("e95")
                nc.vector.tensor_scalar(e95[:], x0[:], 95.0, None, Alu.is_equal)
                s0 = mt("s0", f16)
                s1 = mt("s1", f16)
                tmp = mt("tmp", f16)
                nc.gpsimd.tensor_tensor(s0[:], inr[:], omfx[:], Alu.mult)
                nc.gpsimd.tensor_tensor(tmp[:], em1[:], fx[:], Alu.mult)
                nc.gpsimd.tensor_tensor(s0[:], s0[:], tmp[:], Alu.add)
                nc.gpsimd.tensor_tensor(s1[:], inr[:], fx[:], Alu.mult)
                nc.gpsimd.tensor_tensor(tmp[:], e95[:], omfx[:], Alu.mult)
                nc.gpsimd.tensor_tensor(s1[:], s1[:], tmp[:], Alu.add)

                A0 = mt("A0", f16)
                A1 = mt("A1", f16)
                B0 = mt("B0", f16)
                B1 = mt("B1", f16)
                nc.vector.tensor_tensor(A0[:], wtop[:], s0[:], Alu.mult)
                nc.vector.tensor_tensor(A1[:], wtop[:], s1[:], Alu.mult)
                nc.vector.tensor_tensor(B0[:], wbot[:], s0[:], Alu.mult)
                nc.vector.tensor_tensor(B1[:], wbot[:], s1[:], Alu.mult)

                x0c = mt("x0c")
                nc.vector.tensor_scalar(x0c[:], x0[:], 0.0, 94.0, Alu.max, Alu.min)
                y0c = mt("y0c")
                nc.vector.tensor_scalar(y0c[:], y0[:], 0.0, 95.0, Alu.max, Alu.min)
                y1p = mt("y1p")
                nc.vector.tensor_scalar(y1p[:], y0[:], -1.0, 94.0, Alu.max, Alu.min)
                x0c96 = mt("x0c96")
                nc.vector.tensor_scalar(x0c96[:], x0c[:], 96.0, None, Alu.add)
                IDXT = mt("IDXT")
                nc.vector.scalar_tensor_tensor(IDXT[:], y0c[:], 96.0, x0c[:],
                                               Alu.mult, Alu.add)
                IDXB = mt("IDXB")
                nc.vector.scalar_tensor_tensor(IDXB[:], y1p[:], 96.0, x0c96[:],
                                               Alu.mult, Alu.add)

                # ---------- phase D + per-tap idxw load ----------
                for k in range(K):
                    psw = pbp.tile([NT, 4 * P], f16, tag="psw")
                    for s, tt_ in enumerate((A0, A1, B0, B1)):
                        nc.tensor.transpose(psw[:, s * P:(s + 1) * P],
                                            tt_[:, k * NT:(k + 1) * NT],
                                            ident16[:])
                    evw = pb.tile([NT, 4 * P], f16, tag="evw")
                    nc.scalar.copy(evw[:], psw[:])
                    dstw = AP(tensor=_h(w_rows), offset=(4 * k) * NPOS,
                              ap=[[P, NT], [NPOS, 4], [1, P]])
                    nc.scalar.dma_start(dstw,
                                        evw[:].rearrange("c (s e) -> c s e",
                                                         s=4))

                    psi = pbp.tile([NT, 2 * P], f32, tag="psi")
                    nc.tensor.transpose(psi[:, 0:P],
                                        IDXT[:, k * NT:(k + 1) * NT], ident32[:])
                    nc.tensor.transpose(psi[:, P:2 * P],
                                        IDXB[:, k * NT:(k + 1) * NT], ident32[:])
                    # 16-wrap swizzle on evac: evi[t, h*128 + a*8 + b] =
                    # psi[t, h*128 + b*16 + a]  (a = pix%16, b = (pix%128)//16)
                    evi = pb.tile([NT, 2 * P], i16, tag="evi")
                    nc.vector.tensor_copy(
                        evi[:].rearrange("t (h a b) -> t h a b", h=2, a=16),
                        psi[:].rearrange("t (h b a) -> t h a b", h=2, b=8))
                    # idx_d[a][2k+h][t*8+b] — contiguous wrap-load layout
                    for hh in range(2):
                        dsti = AP(tensor=_h(idx_d), offset=(2 * k + hh) * 576,
                                  ap=[[8, NT], [18 * 576, 16], [1, 8]])
                        nc.sync.dma_start(
                            dsti,
                            evi[:, hh * P:(hh + 1) * P].rearrange(
                                "t (a b) -> t a b", a=16))
                    # wrap-load + replicate once per idx group
                    for gi, (ga, gb) in enumerate(IDX_GROUPS):
                        if k != gb - 1:
                            continue
                        ixg = idxw_g[gi]
                        srcw = AP(tensor=_h(idx_d), offset=(2 * ga) * 576,
                                  ap=[[18 * 576, 16], [1, 1152 * (gb - ga)]])
                        nc.scalar.dma_start(ixg[0:16, :], srcw)
                        nc.scalar.dma_start(ixg[16:32, :], ixg[0:16, :])
                        nc.scalar.dma_start(ixg[32:64, :], ixg[0:32, :])
                        nc.scalar.dma_start(ixg[64:128, :], ixg[0:64, :])

            # ---------- phase F: main loop ----------
            xt_win = AP(tensor=_h(x_t), offset=0, ap=[[P, NROW], [1, 2 * P]])
            with tc.tile_pool(name="g", bufs=4) as gp, \
                 tc.tile_pool(name="aw", bufs=2) as awp, \
                 tc.tile_pool(name="c4", bufs=3) as c4p, \
                 tc.tile_pool(name="wstp", bufs=2) as wsp, \
                 tc.tile_pool(name="ops", bufs=2) as osp, \
                 tc.tile_pool(name="awps", bufs=2, space="PSUM") as awps, \
                 tc.tile_pool(name="outps", bufs=1, space="PSUM") as outps:
                for c in range(NCH):
                    out_ps = outps.tile([P, CW], f32, tag="ops", name="out_ps")
                    c4_prev = []
                    for k in range(K):
                        # DGE inflight ring caps a transpose-gather at
                        # num_idxs*2/16+2 < 128 descriptors per DMA engine,
                        # i.e. num_idxs <= 896 — gather in GW=768 pieces.
                        GW = 768
                        ghs = []
                        for h in range(CW // GW):
                            gTh = gp.tile([P, 2, GW], f16, tag="gT", name="gTh")
                            gBh = gp.tile([P, 2, GW], f16, tag="gB", name="gBh")
                            gi = 0 if k == 0 else (1 if k < 4 else 2)
                            ka = k - IDX_GROUPS[gi][0]
                            i0t = ka * 1152 + (c * CW + h * GW) // 16
                            i0b = ka * 1152 + 576 + (c * CW + h * GW) // 16
                            nc.gpsimd.dma_gather(
                                gTh[:], xt_win,
                                idxw_g[gi][:, i0t:i0t + GW // 16],
                                num_idxs=GW, num_idxs_reg=GW,
                                elem_size=2 * P, elem_step=P, transpose=True)
                            nc.gpsimd.dma_gather(
                                gBh[:], xt_win,
                                idxw_g[gi][:, i0b:i0b + GW // 16],
                                num_idxs=GW, num_idxs_reg=GW,
                                elem_size=2 * P, elem_step=P, transpose=True)
                            ghs.append((gTh, gBh))
                        wst = wsp.tile([1, 4 * CW], f16, tag="wst")
                        wsrc = AP(tensor=_h(w_rows),
                                  offset=(4 * k) * NPOS + c * CW,
                                  ap=[[NPOS, 4], [1, CW]])
                        nc.sync.dma_start(
                            wst[:].rearrange("p (s e) -> p s e", s=4),
                            wsrc.unsqueeze(0))
                        aw = awp.tile([P, 4, CW], f16, tag="aw")
                        awf = aw[:].rearrange("p s c -> p (s c)")
                        for e in range(6):
                            psb = awps.tile([P, 1024], f32, tag="psb", name="psb")
                            base = e * 1024
                            nc.tensor.matmul(psb[:, 0:SUB], ones_row[:],
                                             wst[0:1, base:base + SUB],
                                             start=True, stop=True)
                            nc.tensor.matmul(psb[:, SUB:1024], ones_row[:],
                                             wst[0:1, base + SUB:base + 1024],
                                             start=True, stop=True)
                            if e >= 6 - NEV_DVE:
                                nc.vector.tensor_copy(awf[:, base:base + 1024],
                                                      psb[:])
                            else:
                                nc.scalar.copy(awf[:, base:base + 1024], psb[:])
                        c4 = c4p.tile([P, 4, CW], f16, tag="c4")
                        for h, (gTh, gBh) in enumerate(ghs):
                            hs = slice(h * GW, (h + 1) * GW)
                            nc.vector.tensor_tensor(c4[:, 0:2, hs], gTh[:],
                                                    aw[:, 0:2, hs], Alu.mult)
                            nc.vector.tensor_tensor(c4[:, 2:4, hs], gBh[:],
                                                    aw[:, 2:4, hs], Alu.mult)
                        c4_prev.append((k, c4))
                        # GEMM runs one tap behind so PE never stalls on the
                        # premult: by the time this tap's matmuls issue, the
                        # previous tap's c4 is already finished on DVE.
                        if len(c4_prev) > 1 or k == K - 1:
                            for kg, c4g in list(c4_prev if k == K - 1
                                                else c4_prev[:1]):
                                for j in range(CW // SUB):
                                    for si in range(4):
                                        nc.tensor.matmul(
                                            out_ps[:, j * SUB:(j + 1) * SUB],
                                            WkT[:, kg * P:(kg + 1) * P],
                                            c4g[:, si, j * SUB:(j + 1) * SUB],
                                            start=(kg == 0 and si == 0),
                                            stop=(kg == K - 1 and si == 3),
                                            skip_group_check=True)
                                c4_prev.pop(0)
                    osb = osp.tile([P, CW], f32, tag="osb")
                    if c % 2 == 0:
                        nc.scalar.copy(osb[:], out_ps[:])
                    else:
                        nc.vector.tensor_copy(osb[:], out_ps[:])
                    nc.scalar.dma_start(out[:, c * CW:(c + 1) * CW], osb[:])
    nc.compile()
    return nc


_NC = None


def kernel(x, offset, weight):
    global _NC
    if _NC is None:
        _NC = build_nc()
    from concourse.bass_utils import run_bass_kernel_spmd
    B = x.shape[0]
    w2 = np.ascontiguousarray(weight.reshape(P, 1152)).astype(np.float32)
    in_maps = []
    for b in range(B):
        in_maps.append({
            "x": np.ascontiguousarray(np.asarray(x)[b].reshape(P, NPOS), dtype=np.float32),
            "offset": np.ascontiguousarray(np.asarray(offset)[b].reshape(18, NPOS), dtype=np.float32),
            "weight": w2,
        })
    res = run_bass_kernel_spmd(_NC, in_maps, list(range(B)))
    outs = [res.results[b]["out"].reshape(P, H, W) for b in range(B)]
    return np.stack(outs).astype(np.float32)
